# revision 26
# baseline (speedup 1.0000x reference)
"""Trainium2 Bass kernel for a dense GQA transformer layer (pre-norm, SwiGLU MLP).

Full shapes: B=2, S=2048, H=2048, NH=16, NKV=8, HD=128, FF=5632, fp32 I/O.

Sharding across 8 NeuronCores (one SPMD program):
  core = (b, r) with b = core//4 (data-parallel over batch),
  r = core%4 (sequence-parallel, row-interleaved: core owns rows r::4 of
  batch b). Row interleaving makes the causal-attention work identical on
  every core, which a single SPMD program requires.
  K/V are computed for owned rows only and AllGather'ed (groups of 4).
  Everything else (QKV/O projections, softmax, MLP) is token-parallel with
  full weights per core. Host reassembles the outputs.

Layout strategy: ALL activations live transposed on-chip ([feature, token]
with feature on partitions). Host feeds x already transposed; projections
produce transposed outputs directly (weights stationary, activations
moving); attention computes S^T = K Q^T so softmax probabilities come out
in the layout the PV matmul wants. This eliminates every PE-mode transpose
(the baseline spent ~250us of tensor-engine time on them).

Softmax (no max subtraction; scores are small, masked entries clamped to
-30000 so exp underflows to +0):
  S^T chunks [keys=128, q] -> exp (scalar engine) -> P^T bf16
  denominator = ones-vector matmul over keys (PE), per head
  1/den via DVE reciprocal_approx_fast, broadcast across partitions with a
  K=1 fp32 matmul, applied to the PV output with one DVE multiply.

Precision: bf16 matmuls with fp32 PSUM accumulation; norms, residuals and
softmax denominators in fp32. RMSNorm weights folded into the following
projection weights host-side; weights pre-transposed host-side to [in, out].
"""

import sys

if "/opt/trn_rl_repo" not in sys.path:
    sys.path.insert(0, "/opt/trn_rl_repo")

import math
import os
import numpy as np
import ml_dtypes

import concourse.bass as bass
import concourse.bacc as bacc
import concourse.tile as tile
import concourse.mybir as mybir
from concourse.bass_utils import run_bass_kernel_spmd

F32 = mybir.dt.float32
F32R = mybir.dt.float32r
BF16 = mybir.dt.bfloat16
AFT = mybir.ActivationFunctionType
ALU = mybir.AluOpType

# ---- fixed problem dims ----
B, S, H = 2, 2048, 2048
NH, NKV, HD = 16, 8, 128
FF = 5632
EPS = 1e-6
NC = 8          # cores
TPG = 4         # cores per batch group (sequence-parallel ways)
P = 128         # partitions

MASK_CLAMP = -30000.0
DEBUG = False


def _build_program(S_, FF_, chunk_specs, n_mask):
    """Emit the SPMD program.

    chunk_specs: tuple of (owner, cblk, qlo, (maskadds...)) per key chunk,
    where maskadds are (q_subblock, mask_slot) pairs. qlo is a multiple of
    P. Shared by every core (asserted host-side).
    """
    TOWN = S_ // TPG              # tokens owned per core
    NT = TOWN // P                # token tiles per core
    HT = H // P                   # 16 H tiles
    FC = FF_ // P                 # FF tiles
    KVH = NKV

    assert chunk_specs[0][2] == 0, "first key chunk must cover full q range"

    nc = bacc.Bacc("TRN2", target_bir_lowering=False, debug=False,
                   num_devices=NC)

    # ---- I/O ----
    x_in = nc.dram_tensor("x", [HT, P, TOWN], BF16, kind="ExternalInput").ap()
    wqT = nc.dram_tensor("wqT", [H, NH * HD], BF16, kind="ExternalInput").ap()
    wkT = nc.dram_tensor("wkT", [H, NKV * HD], BF16, kind="ExternalInput").ap()
    wvT = nc.dram_tensor("wvT", [H, NKV * HD], BF16, kind="ExternalInput").ap()
    woT = nc.dram_tensor("woT", [NH * HD, H], BF16, kind="ExternalInput").ap()
    wgT = nc.dram_tensor("wgT", [H, FF_], BF16, kind="ExternalInput").ap()
    wuT = nc.dram_tensor("wuT", [H, FF_], BF16, kind="ExternalInput").ap()
    wdT = nc.dram_tensor("wdT", [FF_, H], BF16, kind="ExternalInput").ap()
    mask_in = nc.dram_tensor("mask", [max(n_mask, 1), P, 2 * P], F32,
                             kind="ExternalInput").ap()
    y_out = nc.dram_tensor("y", [HT, P, TOWN], F32, kind="ExternalOutput").ap()
    if DEBUG:
        dbg_q = nc.dram_tensor("dbg_q", [NH, P, TOWN], BF16,
                               kind="ExternalOutput").ap()
        dbg_a = nc.dram_tensor("dbg_a", [NH, P, TOWN], BF16,
                               kind="ExternalOutput").ap()
        dbg_h1 = nc.dram_tensor("dbg_h1", [HT, P, TOWN], BF16,
                                kind="ExternalOutput").ap()
        dbg_y2 = nc.dram_tensor("dbg_y2", [HT, P, TOWN], BF16,
                                kind="ExternalOutput").ap()
        dbg_m = nc.dram_tensor("dbg_m", [FC, P, TOWN], BF16,
                               kind="ExternalOutput").ap()

    # ---- internal DRAM for the combined K+V AllGather (4 quarters of
    # 2 kv heads each, so attention pipelines behind the gather stream) ----
    KH = KVH // 2
    kv_loc = [nc.dram_tensor(f"kv_loc{i}", [2, 2, P, TOWN], BF16).ap()
              for i in range(4)]
    kv_all = [nc.dram_tensor(f"kv_all{i}", [TPG, 2, 2, P, TOWN], BF16).ap()
              for i in range(4)]

    groups = [[g * TPG + i for i in range(TPG)] for g in range(NC // TPG)]

    from contextlib import ExitStack
    with ExitStack() as ctx:
        tc = ctx.enter_context(tile.TileContext(nc))
        pool = lambda name, bufs, **kw: ctx.enter_context(
            tc.tile_pool(name=name, bufs=bufs, **kw))

        singles = pool("singles", 1)
        x_pool = pool("xp", HT)
        ybuf = pool("ybuf", HT)          # ring reused for Y then Y2
        sq_pool = pool("sqp", 3)
        qT_pool = pool("qTp", NH)
        aT_pool = pool("aTp", NH)
        kT_pool = pool("kTp", 3)
        v_pool = pool("vp", 3)
        esb_pool = pool("esbp", 3)
        mT_pool = pool("mTp", FC)
        cpy_pool = pool("cpyp", 4)
        w512_pool = pool("w512p", 24)
        w256_pool = pool("w256p", 8)
        yout_pool = pool("youtp", 3)
        bc_pool = pool("bcp", 2)
        small_pool = pool("smallp", 6)
        mask_pool = pool("maskp", max(n_mask, 1))

        ones_bf = singles.tile([P, 1], BF16)
        nc.vector.memset(ones_bf, 1.0)
        ones_r1 = singles.tile([1, P], F32)
        nc.vector.memset(ones_r1, 1.0)
        eps_c = singles.tile([1, 1], F32)
        nc.vector.memset(eps_c, EPS)
        eps_one = singles.tile([1, 1], F32)
        nc.vector.memset(eps_one, 1.0)

        x_tiles = []
        for ht in range(HT):
            xt = x_pool.tile([P, TOWN], BF16, tag="x")
            x_tiles.append(xt)
            nc.sync.dma_start(out=xt, in_=x_in[ht])

        mask_sb = []
        for mi in range(n_mask):
            mt = mask_pool.tile([P, 2, P], F32, tag="mask")
            nc.sync.dma_start(out=mt.rearrange("p a b -> p (a b)"),
                              in_=mask_in[mi])
            mask_sb.append(mt)

        def rms_factors(psmall_pool, pbc_pool):
            """-> bc [P, TOWN] f32 (rstd broadcast along partitions) and
            rstd [1, TOWN] f32, from the current x_tiles."""
            ssum = psmall_pool.tile([1, TOWN], F32, tag="pss")
            for ht in range(HT):
                sq = sq_pool.tile([P, TOWN], BF16, tag="sq")
                nc.scalar.activation(out=sq, in_=x_tiles[ht], func=AFT.Square)
                nc.tensor.matmul(ssum, lhsT=ones_bf, rhs=sq,
                                 start=(ht == 0), stop=(ht == HT - 1))
            std = small_pool.tile([1, TOWN], F32, tag="std", bufs=2)
            nc.scalar.activation(out=std, in_=ssum, func=AFT.Sqrt,
                                 scale=1.0 / H, bias=eps_c)
            rstd = small_pool.tile([1, TOWN], F32, tag="rstd", bufs=2)
            nc.vector.reciprocal_approx_fast(out=rstd, in_=std)
            bc_ps = pbc_pool.tile([P, TOWN], F32, tag="pbc", bufs=1)
            nc.tensor.matmul(bc_ps, lhsT=ones_r1, rhs=rstd,
                             start=True, stop=True)
            bc = bc_pool.tile([P, TOWN], F32, tag="bc")
            nc.vector.tensor_copy(bc, bc_ps)
            return bc, rstd

        # ================= phase 1: norms + projections =================
        with tc.tile_pool(name="pmm", bufs=5, space="PSUM") as pmm, \
             tc.tile_pool(name="psmall", bufs=1, space="PSUM") as psmall:

            bc1, rstd1 = rms_factors(psmall, pmm)
            # rstd transposed to [tokens, 1] slices for the V drain
            prT = psmall.tile([P, NT], F32, tag="prT", bufs=1)
            for t in range(NT):
                nc.tensor.matmul(prT[:, t:t + 1],
                                 lhsT=rstd1[:, t * P:(t + 1) * P],
                                 rhs=eps_one, start=True, stop=True)
            rstdT = small_pool.tile([P, NT], F32, tag="rstdT", bufs=1)
            nc.scalar.activation(out=rstdT, in_=prT, func=AFT.Copy)

            # ---- K/V projections + gathers in quarters of 2 kv heads:
            # each quarter's gather triggers as soon as its K and V are
            # computed, so the serialized collective stream starts early and
            # attention pipelines behind it ----
            for q in range(4):
                pk = [pmm.tile([P, TOWN], F32, tag="pmm", name=f"pk{j}")
                      for j in range(2)]
                for ht in range(HT):
                    wl = w256_pool.tile([P, 256], BF16, tag="w256")
                    nc.sync.dma_start(
                        out=wl, in_=wkT[ht * P:(ht + 1) * P,
                                        q * 256:(q + 1) * 256])
                    for j in range(2):
                        nc.tensor.matmul(pk[j], lhsT=wl[:, j * P:(j + 1) * P],
                                         rhs=x_tiles[ht], start=(ht == 0),
                                         stop=(ht == HT - 1))
                for j in range(2):
                    kc = cpy_pool.tile([P, TOWN], BF16, tag="cpy")
                    nc.vector.tensor_mul(kc, pk[j], bc1)
                    nc.sync.dma_start(out=kv_loc[q][0, j], in_=kc)

                pv = [pmm.tile([P, 2 * HD], F32, tag="pmm", name=f"pv{t}")
                      for t in range(NT)]
                for ht in range(HT):
                    wv = w256_pool.tile([P, 256], BF16, tag="w256")
                    nc.sync.dma_start(
                        out=wv, in_=wvT[ht * P:(ht + 1) * P,
                                        q * 256:(q + 1) * 256])
                    for t in range(NT):
                        nc.tensor.matmul(pv[t],
                                         lhsT=x_tiles[ht][:, t * P:(t + 1) * P],
                                         rhs=wv, start=(ht == 0),
                                         stop=(ht == HT - 1))
                for t in range(NT):
                    vc = cpy_pool.tile([P, 2, HD], BF16, tag="cpy")
                    nc.scalar.activation(
                        out=vc.rearrange("p k d -> p (k d)"), in_=pv[t],
                        func=AFT.Copy, scale=rstdT[:, t:t + 1])
                    nc.sync.dma_start(
                        out=kv_loc[q][1].rearrange(
                            "k p (t d) -> k p t d", d=HD)[
                            :, :, t, :].rearrange("k p d -> p k d"),
                        in_=vc)
                nc.gpsimd.collective_compute(
                    "AllGather", ALU.bypass, ins=[kv_loc[q].opt()],
                    outs=[kv_all[q].opt()], replica_groups=groups)

            # ---- Q projection (transposed out, scale folded in weights) ----
            qT = []
            for g in range(4):
                pq = [pmm.tile([P, TOWN], F32, tag="pmm", name=f"pq{j}")
                      for j in range(4)]
                for ht in range(HT):
                    wq = w512_pool.tile([P, 512], BF16, tag="w512")
                    nc.sync.dma_start(
                        out=wq, in_=wqT[ht * P:(ht + 1) * P,
                                        g * 512:(g + 1) * 512])
                    for j in range(4):
                        nc.tensor.matmul(pq[j], lhsT=wq[:, j * P:(j + 1) * P],
                                         rhs=x_tiles[ht], start=(ht == 0),
                                         stop=(ht == HT - 1))
                for j in range(4):
                    qt = qT_pool.tile([P, TOWN], BF16, tag="qT")
                    nc.vector.tensor_mul(qt, pq[j], bc1)
                    if DEBUG:
                        nc.sync.dma_start(out=dbg_q[g * 4 + j], in_=qt)
                    qT.append(qt)

        # ================= phase 2: attention =================
        aT = []
        with tc.tile_pool(name="psc", bufs=2, space="PSUM") as psc, \
             tc.tile_pool(name="pav", bufs=2, space="PSUM") as pav, \
             tc.tile_pool(name="pden", bufs=2, space="PSUM") as pden:
            nchunks = len(chunk_specs)
            for kvh in range(KVH):
                g, kk = kvh // 2, kvh % 2
                kT_sb = kT_pool.tile([P, TPG, TOWN], BF16, tag="kT")
                for o in range(TPG):
                    nc.sync.dma_start(out=kT_sb[:, o, :],
                                      in_=kv_all[g][o, 0, kk])
                v_sb = v_pool.tile([P, TPG, NT, HD], BF16, tag="vT")
                for o in range(TPG):
                    nc.sync.dma_start(
                        out=v_sb[:, o].rearrange("p t d -> p (t d)"),
                        in_=kv_all[g][o, 1, kk])

                aT_ps = [pav.tile([P, TOWN], F32, tag="pav", name=f"pa{j}")
                         for j in range(2)]
                den_ps = [pden.tile([1, TOWN], F32, tag="pden", name=f"pd{j}")
                          for j in range(2)]

                def emit_av(spec, esb, idx):
                    o, c, qlo, _ = spec
                    for h2 in range(2):
                        nc.tensor.matmul(
                            aT_ps[h2][:, qlo:], lhsT=v_sb[:, o, c, :],
                            rhs=esb[:, h2, qlo:], start=(idx == 0),
                            stop=(idx == nchunks - 1))
                    for h2 in range(2):
                        nc.tensor.matmul(
                            den_ps[h2][:, qlo:], lhsT=ones_bf,
                            rhs=esb[:, h2, qlo:], start=(idx == 0),
                            stop=(idx == nchunks - 1))

                pending = None
                for i, spec in enumerate(chunk_specs):
                    o, c, qlo, madds = spec
                    sc = psc.tile([P, 2, TOWN], F32, tag="psc",
                                  padded_shape=[P, 2, 512])
                    for h2 in range(2):
                        nc.tensor.matmul(
                            sc[:, h2, qlo:],
                            lhsT=kT_sb[:, o, c * P:(c + 1) * P],
                            rhs=qT[2 * kvh + h2][:, qlo:],
                            start=True, stop=True)
                    for qb, mi in madds:
                        off = qb * P
                        nc.vector.tensor_add(sc[:, :, off:off + P],
                                             sc[:, :, off:off + P],
                                             mask_sb[mi])
                    esb = esb_pool.tile([P, 2, TOWN], BF16, tag="esb")
                    nc.scalar.activation(out=esb[:, :, qlo:],
                                         in_=sc[:, :, qlo:], func=AFT.Exp)
                    if pending is not None:
                        emit_av(*pending)
                    pending = (spec, esb, i)
                emit_av(*pending)

                for h2 in range(2):
                    rec = small_pool.tile([1, TOWN], F32, tag="rec", bufs=2)
                    nc.vector.reciprocal_approx_fast(out=rec, in_=den_ps[h2])
                    bcp = psc.tile([P, 2, TOWN], F32, tag="psc",
                                   padded_shape=[P, 2, 512])
                    nc.tensor.matmul(bcp[:, 0, :], lhsT=ones_r1, rhs=rec,
                                     start=True, stop=True)
                    bcs = bc_pool.tile([P, TOWN], F32, tag="bc")
                    nc.vector.tensor_copy(bcs, bcp[:, 0, :])
                    at = aT_pool.tile([P, TOWN], BF16, tag="aT")
                    nc.vector.tensor_mul(at, aT_ps[h2], bcs)
                    if DEBUG:
                        nc.sync.dma_start(out=dbg_a[2 * kvh + h2], in_=at)
                    aT.append(at)

        # ================= phase 3: O proj + residual + rmsnorm2 =========
        with tc.tile_pool(name="pmm2", bufs=5, space="PSUM") as pmm2, \
             tc.tile_pool(name="psmall2", bufs=1, space="PSUM") as psmall2:
            for hg in range(4):
                po = [pmm2.tile([P, TOWN], F32, tag="pmm2", name=f"po{j}")
                      for j in range(4)]
                for h in range(NH):
                    wo = w512_pool.tile([P, 512], BF16, tag="w512")
                    nc.sync.dma_start(
                        out=wo, in_=woT[h * P:(h + 1) * P,
                                        hg * 512:(hg + 1) * 512])
                    for j in range(4):
                        nc.tensor.matmul(po[j], lhsT=wo[:, j * P:(j + 1) * P],
                                         rhs=aT[h], start=(h == 0),
                                         stop=(h == NH - 1))
                for j in range(4):
                    hc = hg * 4 + j
                    nc.vector.tensor_add(x_tiles[hc], x_tiles[hc], po[j])
                    if DEBUG:
                        nc.sync.dma_start(out=dbg_h1[hc], in_=x_tiles[hc])

            bc2, _ = rms_factors(psmall2, pmm2)
            Y2 = []
            for ht in range(HT):
                yt = ybuf.tile([P, TOWN], BF16, tag="y")
                nc.vector.tensor_mul(yt, x_tiles[ht], bc2)
                Y2.append(yt)
            if DEBUG:
                for ht in range(HT):
                    nc.sync.dma_start(out=dbg_y2[ht], in_=Y2[ht])

        # ================= phase 4: MLP =================
        mT = []
        with tc.tile_pool(name="pgate", bufs=1, space="PSUM") as pgate, \
             tc.tile_pool(name="pup", bufs=1, space="PSUM") as pup:
            for fg in range(FC // 4):
                pg = pgate.tile([P, 4, TOWN], F32, tag="pgate",
                                padded_shape=[P, 4, 512])
                for ht in range(HT):
                    wg = w512_pool.tile([P, 512], BF16, tag="w512")
                    nc.sync.dma_start(
                        out=wg, in_=wgT[ht * P:(ht + 1) * P,
                                        fg * 512:(fg + 1) * 512])
                    for j in range(4):
                        nc.tensor.matmul(pg[:, j, :],
                                         lhsT=wg[:, j * P:(j + 1) * P],
                                         rhs=Y2[ht], start=(ht == 0),
                                         stop=(ht == HT - 1))
                pu = pup.tile([P, 4, TOWN], F32, tag="pup",
                              padded_shape=[P, 4, 512])
                for ht in range(HT):
                    wu = w512_pool.tile([P, 512], BF16, tag="w512")
                    nc.sync.dma_start(
                        out=wu, in_=wuT[ht * P:(ht + 1) * P,
                                        fg * 512:(fg + 1) * 512])
                    for j in range(4):
                        nc.tensor.matmul(pu[:, j, :],
                                         lhsT=wu[:, j * P:(j + 1) * P],
                                         rhs=Y2[ht], start=(ht == 0),
                                         stop=(ht == HT - 1))
                for j in range(4):
                    sg = cpy_pool.tile([P, TOWN], BF16, tag="cpy")
                    nc.scalar.activation(out=sg, in_=pg[:, j, :],
                                         func=AFT.Silu)
                    mt = mT_pool.tile([P, TOWN], BF16, tag="mT")
                    nc.vector.tensor_mul(mt, sg, pu[:, j, :])
                    if DEBUG:
                        nc.sync.dma_start(out=dbg_m[fg * 4 + j], in_=mt)
                    mT.append(mt)

        with tc.tile_pool(name="pdown", bufs=2, space="PSUM") as pdown:
            for hg in range(HT // 4):
                pd = pdown.tile([P, 4, TOWN], F32, tag="pdown",
                                padded_shape=[P, 4, 512])
                for fc in range(FC):
                    wd = w512_pool.tile([P, 512], BF16, tag="w512")
                    nc.sync.dma_start(
                        out=wd, in_=wdT[fc * P:(fc + 1) * P,
                                        hg * 512:(hg + 1) * 512])
                    for j in range(4):
                        nc.tensor.matmul(pd[:, j, :],
                                         lhsT=wd[:, j * P:(j + 1) * P],
                                         rhs=mT[fc], start=(fc == 0),
                                         stop=(fc == FC - 1))
                for j in range(4):
                    hc = hg * 4 + j
                    yo = yout_pool.tile([P, TOWN], F32, tag="yout")
                    nc.vector.tensor_add(yo, x_tiles[hc], pd[:, j, :])
                    nc.sync.dma_start(out=y_out[hc], in_=yo)

    nc.compile()
    return nc


_CACHE = {}
LAST_RESULT = None


def _get_program(S_, FF_, chunk_specs, n_mask):
    key = (S_, FF_, chunk_specs, n_mask)
    if key not in _CACHE:
        _CACHE[key] = _build_program(S_, FF_, chunk_specs, n_mask)
    return _CACHE[key]


def _prep_weights(q_w, k_w, v_w, o_w, gate_w, up_w, down_w, ln1_w, ln2_w):
    bf = ml_dtypes.bfloat16
    wqT = np.ascontiguousarray(
        (q_w * ln1_w[None, :]).T * (1.0 / math.sqrt(HD))).astype(bf)
    wkT = np.ascontiguousarray((k_w * ln1_w[None, :]).T).astype(bf)
    wvT = np.ascontiguousarray((v_w * ln1_w[None, :]).T).astype(bf)
    woT = np.ascontiguousarray(o_w.T).astype(bf)
    wgT = np.ascontiguousarray((gate_w * ln2_w[None, :]).T).astype(bf)
    wuT = np.ascontiguousarray((up_w * ln2_w[None, :]).T).astype(bf)
    wdT = np.ascontiguousarray(down_w.T).astype(bf)
    return wqT, wkT, wvT, woT, wgT, wuT, wdT


def _mask_structure_T(m, S_):
    """Derive the shared chunk structure + per-core mask blocks.

    m: clamped [S, S] additive mask (query, key).
    Returns (chunk_specs, n_mask, blocks) where blocks[core][slot] is a
    [P, P] (key, query) fp32 block.
    """
    TOWN = S_ // TPG
    NTc = TOWN // P
    specs_per_core = []
    blocks_per_core = []
    for r in range(TPG):
        qg = r + TPG * np.arange(TOWN)
        spec_r = []
        blocks_r = {}
        for c in range(NTc):
            for o in range(TPG):
                kg = o + TPG * (c * P + np.arange(P))
                sub = m[np.ix_(qg, kg)]                # [TOWN, P] (q, k)
                allowed = sub > MASK_CLAMP
                qany = allowed.any(axis=1)
                if not qany.any():
                    continue
                qlo = int(np.argmax(qany)) // P * P
                madds = []
                for qb in range(qlo // P, TOWN // P):
                    blk = sub[qb * P:(qb + 1) * P, :]
                    if (blk < 0).any():
                        madds.append(qb)
                        blocks_r[(o, c, qb)] = np.ascontiguousarray(
                            blk.T).astype(np.float32)
                spec_r.append((o, c, qlo, tuple(madds)))
        specs_per_core.append(tuple(spec_r))
        blocks_per_core.append(blocks_r)
    assert all(s == specs_per_core[0] for s in specs_per_core), \
        "mask structure must be identical across cores (SPMD)"
    raw_spec = specs_per_core[0]

    # Dedupe mask blocks: two (o,c,qb) keys share a slot only if their
    # content matches on EVERY core.
    slot_of = {}
    sig_to_slot = {}
    for (o, c, qlo, madds) in raw_spec:
        for qb in madds:
            key = (o, c, qb)
            sig = tuple(blocks_per_core[r][key].tobytes()
                        for r in range(TPG))
            if sig not in sig_to_slot:
                sig_to_slot[sig] = len(sig_to_slot)
            slot_of[key] = sig_to_slot[sig]
    n_mask = len(sig_to_slot)

    chunk_specs = tuple(
        (o, c, qlo, tuple((qb, slot_of[(o, c, qb)]) for qb in madds))
        for (o, c, qlo, madds) in raw_spec)
    # order chunks so the first covers the full q range (PSUM init)
    chunk_specs = tuple(sorted(chunk_specs, key=lambda t: (t[2], t[1], t[0])))

    blocks = []
    for r in range(TPG):
        blk_list = [None] * n_mask
        for key, slot in slot_of.items():
            if blk_list[slot] is None:
                blk_list[slot] = blocks_per_core[r][key]
        blocks.append(blk_list)
    return chunk_specs, n_mask, blocks


def kernel(hidden_states, attention_mask, q_w, k_w, v_w, o_w,
           gate_w, up_w, down_w, ln1_w, ln2_w):
    hidden_states = np.asarray(hidden_states, np.float32)
    m = np.maximum(np.asarray(attention_mask, np.float32)[0, 0], MASK_CLAMP)
    S_ = hidden_states.shape[1]
    FF_ = gate_w.shape[0]
    TOWN = S_ // TPG
    HT = H // P

    chunk_specs, n_mask, blocks = _mask_structure_T(m, S_)
    nc = _get_program(S_, FF_, chunk_specs, n_mask)

    wqT, wkT, wvT, woT, wgT, wuT, wdT = _prep_weights(
        np.asarray(q_w, np.float32), np.asarray(k_w, np.float32),
        np.asarray(v_w, np.float32), np.asarray(o_w, np.float32),
        np.asarray(gate_w, np.float32), np.asarray(up_w, np.float32),
        np.asarray(down_w, np.float32), np.asarray(ln1_w, np.float32),
        np.asarray(ln2_w, np.float32))

    in_maps = []
    for core in range(NC):
        b, r = core // TPG, core % TPG
        rows = r + TPG * np.arange(TOWN)
        xT = np.ascontiguousarray(hidden_states[b, rows].T).reshape(
            HT, P, TOWN).astype(ml_dtypes.bfloat16)
        mask_blocks = np.zeros((max(n_mask, 1), P, 2 * P), np.float32)
        for slot in range(n_mask):
            blk = blocks[r][slot]
            mask_blocks[slot, :, :P] = blk
            mask_blocks[slot, :, P:] = blk
        in_maps.append({
            "x": xT, "mask": mask_blocks,
            "wqT": wqT, "wkT": wkT, "wvT": wvT, "woT": woT,
            "wgT": wgT, "wuT": wuT, "wdT": wdT,
        })

    res = run_bass_kernel_spmd(nc, in_maps, list(range(NC)),
                               trace=bool(os.environ.get("KERNEL_TRACE")))
    global LAST_RESULT
    LAST_RESULT = res

    out = np.empty((B, S_, H), np.float32)
    for core in range(NC):
        b, r = core // TPG, core % TPG
        rows = r + TPG * np.arange(TOWN)
        yT = res.results[core]["y"].reshape(H, TOWN)
        out[b, rows] = yT.T
    return out
